# revision 10
# baseline (speedup 1.0000x reference)
"""Trainium2 Bass kernel for nn_ClusterGCN (3-layer 2-edge-type GCN + heads).

Strategy (8 NeuronCores, node-parallel):
  - Nodes sharded contiguously: core c owns rows [c*S, (c+1)*S), S = N/8.
  - Per layer, each core holds a replicated node-major bf16 table of
    h_tilde_t = dinv_t * h (one per edge type) in DRAM; edges are sharded by
    dst. Messages h_tilde[src] are fetched with GPSIMD dma_gather (int16
    indices, lo/hi base split for N > 32768) and scatter-added into a
    feature-major accumulator via one-hot matmuls on the PE
    (out[feat, dst_slot] += msg[edge, feat]^T @ onehot[edge, dst_slot]).
  - g_t = dinv_t * (scatter + dinv_t * h) adds the self-loop, then
    h' = BN(g0 @ W0 + g1 @ W1) with batch stats AllReduced across cores.
  - Tables for the next layer are rebuilt feature-major, PE-transposed to
    node-major, and AllGathered.  Heads (tanh/relu/l2norm MLPs) run
    node-sharded at the end.
"""
import math
import numpy as np
import ml_dtypes

import concourse.bacc as bacc
import concourse.bass as bass
import concourse.mybir as mybir
import concourse.tile as tile
from concourse.library_config import mlp as mlp_lib
from concourse.bass_utils import run_bass_kernel_spmd

NCORES = 8
D = 128
L = 3
EPS_BN = 1e-5
EPS_NORM = 1e-12
IDX_LIMIT = 32768
PIECE = 1024          # gather slots per dma_gather instruction
SENT_DST = 320.0      # sentinel dst slot (bf16-exact, >= 128)

f32 = mybir.dt.float32
bf16 = mybir.dt.float16  # (fp16 everywhere: 8x finer mantissa than bf16, same HW rates)
i16 = mybir.dt.int16
AF = mybir.ActivationFunctionType
OP = mybir.AluOpType


# ---------------------------------------------------------------- host prep

def _prep_type(edge_index, N, S):
    """Per edge type: degrees + per-core common-shape gather/scatter schedule."""
    src = np.asarray(edge_index[0], np.int64)
    dst = np.asarray(edge_index[1], np.int64)
    deg = np.bincount(dst, minlength=N).astype(np.float32) + 1.0
    dinv = (1.0 / np.sqrt(deg)).astype(np.float32)

    NW = (S + 127) // 128
    cores = []
    for c in range(NCORES):
        m = (dst >= c * S) & (dst < (c + 1) * S)
        s_c = src[m]
        dl = dst[m] - c * S
        w = dl // 128
        reg = (s_c >= IDX_LIMIT).astype(np.int64)
        order = np.lexsort((dl, w, reg))
        cores.append((s_c[order], dl[order], w[order], reg[order]))

    # cross-core max chunk count per (region, window) -> common SPMD schedule
    K = np.zeros((2, NW), np.int64)
    for (s_c, dl, w, reg) in cores:
        for r in (0, 1):
            cnt = np.bincount(w[reg == r], minlength=NW)
            K[r] = np.maximum(K[r], (cnt + 127) // 128)

    schedule = []  # (region, window, nchunks) in slot order
    for r in (0, 1):
        for w_ in range(NW):
            if K[r][w_] > 0:
                schedule.append((r, int(w_), int(K[r][w_])))
    nchunks = sum(k for _, _, k in schedule)
    stot = nchunks * 128

    idx_all = np.zeros((NCORES, max(stot, 128)), np.int64)
    rel_all = np.full((NCORES, max(nchunks, 1) * 128), SENT_DST, np.float64)
    for ci, (s_c, dl, w, reg) in enumerate(cores):
        pos = 0
        for (r, w_, k) in schedule:
            m = (reg == r) & (w == w_)
            n = int(m.sum())
            sv = s_c[m]
            idx_all[ci, pos:pos + n] = np.where(r == 0, sv, sv - (N - IDX_LIMIT))
            rel_all[ci, pos:pos + n] = dl[m] - w_ * 128
            pos += k * 128

    # wrapped int16 index layout: idxs[p, s] = idx[s*16 + p%16]
    cols = max(stot // 16, 1)
    idx_w = np.zeros((NCORES, 128, cols), np.int16)
    if stot:
        a = idx_all[:, :stot].reshape(NCORES, cols, 16)  # [c, s, j]
        for p in range(128):
            idx_w[:, p, :] = a[:, :, p % 16]
    # dst-slot tile: rel[p, chunk] = slot of edge chunk*128+p
    rel_t = np.ascontiguousarray(
        rel_all[:, :nchunks * 128].reshape(NCORES, nchunks, 128).transpose(0, 2, 1)
    ).astype(np.float16)

    return dinv, schedule, idx_w, rel_t, stot, nchunks


def _pieces(schedule):
    """Split slot range into gather pieces that do not cross the lo/hi boundary.
    Returns list of (slot_start, slot_count, region)."""
    out = []
    for r in (0, 1):
        lo = sum(k * 128 for (rr, _, k) in schedule if rr < r)
        n = sum(k * 128 for (rr, _, k) in schedule if rr == r)
        p = lo
        while p < lo + n:
            c = min(PIECE, lo + n - p)
            out.append((p, c, r))
            p += c
    return out


# ---------------------------------------------------------------- device build

def _build(N, S, sch0, stot0, nch0, sch1, stot1, nch1):
    NW = (S + 127) // 128
    NF = (S + 511) // 512  # 512-wide node tiles
    nc = bacc.Bacc("TRN2", target_bir_lowering=False, debug=False,
                   num_devices=NCORES)

    def din(name, shape, dt):
        return nc.dram_tensor(name, shape, dt, kind="ExternalInput")

    tab_in = [din("tab0_in", [N, D], bf16), din("tab1_in", [N, D], bf16)]
    xT_in = din("xT_in", [128, S], f32)
    dinv_in = [din("dinv0_in", [128, S], bf16), din("dinv1_in", [128, S], bf16)]
    idx_in = [din("idx0_in", [128, max(stot0 // 16, 1)], i16),
              din("idx1_in", [128, max(stot1 // 16, 1)], i16)]
    rel_in = [din("rel0_in", [128, max(nch0, 1)], bf16),
              din("rel1_in", [128, max(nch1, 1)], bf16)]
    wd_in = din("wd_in", [L * 2 * 128, D], bf16)
    gb_in = din("gb_in", [128, 2 * L], f32)
    wh_in = din("wh_in", [6 * 128, D], bf16)
    hb_in = din("hb_in", [128, 6], f32)
    iota_in = din("iota_in", [128, 128], bf16)
    ident_in = din("ident_in", [128, 128], bf16)
    ones_in = din("ones_in", [128, 128], bf16)

    outs = [nc.dram_tensor(n, [128, S], f32, kind="ExternalOutput")
            for n in ("e1_o", "e2_o", "p1_o", "p2_o")]

    with tile.TileContext(nc) as tc:
        with (
            tc.tile_pool(name="const", bufs=1) as const,
            tc.tile_pool(name="g", bufs=1) as gpool,
            tc.tile_pool(name="msg", bufs=2) as msgp,
            tc.tile_pool(name="oh", bufs=6) as ohp,
            tc.tile_pool(name="scr", bufs=1) as scp,
            tc.tile_pool(name="psA", bufs=4, space="PSUM") as psA,
            tc.tile_pool(name="psT", bufs=2, space="PSUM") as psT,
            tc.tile_pool(name="psB", bufs=2, space="PSUM") as psB,
            tc.tile_pool(name="dram", bufs=1, space="DRAM") as dram,
        ):
            nc.gpsimd.load_library(mlp_lib)

            # ---- persistent SBUF tiles
            iota_t = const.tile([128, 128], bf16)
            ident_t = const.tile([128, 128], bf16)
            ones_t = const.tile([128, 128], bf16)
            dinv_t = [const.tile([128, S], bf16, tag=f"dinv{t}", name=f"dinv{t}") for t in (0, 1)]
            idx_t = [const.tile([128, max(stot0 // 16, 1)], i16, tag="idx0", name="idx0"),
                     const.tile([128, max(stot1 // 16, 1)], i16, tag="idx1", name="idx1")]
            rel_t = [const.tile([128, max(nch0, 1)], bf16, tag="rel0", name="rel0"),
                     const.tile([128, max(nch1, 1)], bf16, tag="rel1", name="rel1")]
            wd_t = const.tile([128, L * 2, D], bf16)     # dense weights
            wh_t = const.tile([128, 6, D], bf16)         # head weights
            gb_t = const.tile([128, 2 * L], f32)
            hb_t = const.tile([128, 6], f32)

            h_t = gpool.tile([128, S], f32, tag="h")
            g_t = [gpool.tile([128, S], f32, tag=f"g{t}", name=f"g{t}") for t in (0, 1)]
            gbf_t = [gpool.tile([128, S], bf16, tag=f"gbf{t}", name=f"gbf{t}") for t in (0, 1)]

            nc.sync.dma_start(iota_t[:], iota_in[:])
            nc.sync.dma_start(ident_t[:], ident_in[:])
            nc.sync.dma_start(ones_t[:], ones_in[:])
            for t in (0, 1):
                nc.sync.dma_start(dinv_t[t][:], dinv_in[t][:])
                nc.sync.dma_start(idx_t[t][:], idx_in[t][:])
                nc.sync.dma_start(rel_t[t][:], rel_in[t][:])
            nc.sync.dma_start(
                wd_t[:], wd_in[:].rearrange("(k p) d -> p k d", p=128))
            nc.sync.dma_start(
                wh_t[:], wh_in[:].rearrange("(k p) d -> p k d", p=128))
            nc.sync.dma_start(gb_t[:], gb_in[:])
            nc.sync.dma_start(hb_t[:], hb_in[:])
            nc.sync.dma_start(h_t[:], xT_in[:])

            # ---- internal DRAM for collectives
            ag_in = {}
            ag_out = {}
            for l in (0, 1):
                for t in (0, 1):
                    ag_in[(l, t)] = dram.tile([S, D], bf16, tag=f"agi{l}{t}", name=f"agi{l}{t}")
                    ag_out[(l, t)] = dram.tile([N, D], bf16,
                                               addr_space="Shared",
                                               tag=f"ago{l}{t}",
                                               name=f"ago{l}{t}")
            st_in = [dram.tile([128, 2], f32, tag=f"sti{l}", name=f"sti{l}") for l in range(L)]
            st_out = [dram.tile([128, 2], f32, addr_space="Shared",
                                tag=f"sto{l}", name=f"sto{l}") for l in range(L)]

            schs = (sch0, sch1)
            rg = [list(range(NCORES))]

            for l in range(L):
                # ---------------- scatter phase (both edge types)
                for t in (0, 1):
                    # g init: self-loop inner term  g = dinv * h
                    nc.vector.tensor_tensor(out=g_t[t][:], in0=h_t[:],
                                            in1=dinv_t[t][:], op=OP.mult)
                    if l == 0:
                        tab_lo = tab_in[t][:]
                        tab_hi = tab_in[t][N - IDX_LIMIT:] if N > IDX_LIMIT else None
                    else:
                        tab_lo = ag_out[(l - 1, t)][:]
                        tab_hi = ag_out[(l - 1, t)][N - IDX_LIMIT:] \
                            if N > IDX_LIMIT else None

                    sch = schs[t]
                    # chunk -> (region, window, first?, last?) map
                    chunk_meta = []
                    for (r, w_, k) in sch:
                        for j in range(k):
                            chunk_meta.append((r, w_, j == 0, j == k - 1))

                    pieces = _pieces(sch)
                    acc = None
                    for (p0, cnt, r) in pieces:
                        msg = msgp.tile([128, PIECE // 128, 128], bf16,
                                        tag="msg")
                        src_ap = tab_lo if r == 0 else tab_hi
                        nc.gpsimd.dma_gather(
                            msg[:, :cnt // 128, :], src_ap,
                            idx_t[t][:, p0 // 16:(p0 + cnt) // 16],
                            num_idxs=cnt, num_idxs_reg=cnt, elem_size=D,
                        )
                        for ci in range(cnt // 128):
                            gc = p0 // 128 + ci
                            (cr, w_, first, last) = chunk_meta[gc]
                            oh = ohp.tile([128, 128], bf16, tag="oh")
                            nc.vector.tensor_tensor(
                                out=oh[:],
                                in0=rel_t[t][:, gc:gc + 1].to_broadcast([128, 128]),
                                in1=iota_t[:],
                                op=OP.is_equal,
                            )
                            if first:
                                acc = psA.tile([128, 128], f32, space="PSUM",
                                               tag="sc")
                            nc.tensor.matmul(out=acc[:], lhsT=msg[:, ci, :],
                                             rhs=oh[:], start=first, stop=last)
                            if last:
                                wd = min(128, S - w_ * 128)
                                sl = slice(w_ * 128, w_ * 128 + wd)
                                nc.vector.tensor_tensor(
                                    out=g_t[t][:, sl], in0=g_t[t][:, sl],
                                    in1=acc[:, :wd], op=OP.add)
                    # final dst-side scale, bf16 for dense matmul
                    nc.vector.tensor_tensor(out=gbf_t[t][:], in0=g_t[t][:],
                                            in1=dinv_t[t][:], op=OP.mult)

                # ---------------- dense + stats partials
                sum_p = scp.tile([128, NF], f32, tag="sump")
                ssq_p = scp.tile([128, NF], f32, tag="ssqp")
                for ft in range(NF):
                    fw = min(512, S - ft * 512)
                    sl = slice(ft * 512, ft * 512 + fw)
                    dp = psB.tile([128, 512], f32, space="PSUM", tag="dense")
                    nc.tensor.matmul(out=dp[:, :fw], lhsT=wd_t[:, l * 2, :],
                                     rhs=gbf_t[0][:, sl], start=True, stop=False)
                    nc.tensor.matmul(out=dp[:, :fw], lhsT=wd_t[:, l * 2 + 1, :],
                                     rhs=gbf_t[1][:, sl], start=False, stop=True)
                    nc.vector.tensor_reduce(out=sum_p[:, ft:ft + 1],
                                            in_=dp[:, :fw],
                                            axis=mybir.AxisListType.X,
                                            op=OP.add)
                    sq = scp.tile([128, 512], f32, tag="sq", bufs=2)
                    nc.scalar.activation(out=sq[:, :fw], in_=dp[:, :fw],
                                         func=AF.Square,
                                         accum_out=ssq_p[:, ft:ft + 1])
                    nc.vector.tensor_copy(out=h_t[:, sl], in_=dp[:, :fw])

                # ---------------- BN stats allreduce
                st = scp.tile([128, 2], f32, tag="st")
                nc.vector.tensor_reduce(out=st[:, 0:1], in_=sum_p[:],
                                        axis=mybir.AxisListType.X, op=OP.add)
                nc.vector.tensor_reduce(out=st[:, 1:2], in_=ssq_p[:],
                                        axis=mybir.AxisListType.X, op=OP.add)
                nc.sync.dma_start(st_in[l][:], st[:])
                nc.gpsimd.collective_compute(
                    "AllReduce", OP.add, replica_groups=rg,
                    ins=[st_in[l].opt()], outs=[st_out[l].opt()])
                sta = scp.tile([128, 2], f32, tag="sta")
                nc.sync.dma_start(sta[:], st_out[l][:])

                mean = scp.tile([128, 1], f32, tag="mean")
                var = scp.tile([128, 1], f32, tag="var")
                scl = scp.tile([128, 1], f32, tag="scl")
                sht = scp.tile([128, 1], f32, tag="sht")
                tmp = scp.tile([128, 1], f32, tag="tmp1")
                inv_n = 1.0 / float(N)
                nc.vector.tensor_scalar(out=mean[:], in0=sta[:, 0:1],
                                        scalar1=inv_n, scalar2=None, op0=OP.mult)
                nc.vector.tensor_scalar(out=var[:], in0=sta[:, 1:2],
                                        scalar1=inv_n, scalar2=None, op0=OP.mult)
                nc.vector.tensor_tensor(out=tmp[:], in0=mean[:], in1=mean[:],
                                        op=OP.mult)
                nc.vector.tensor_tensor(out=var[:], in0=var[:], in1=tmp[:],
                                        op=OP.subtract)
                # scl = gamma / sqrt(var + eps); sht = beta - mean*scl
                nc.vector.tensor_scalar(out=var[:], in0=var[:], scalar1=EPS_BN,
                                        scalar2=None, op0=OP.add)
                nc.scalar.activation(out=tmp[:], in_=var[:], func=AF.Sqrt)
                nc.vector.reciprocal(out=tmp[:], in_=tmp[:])
                nc.vector.tensor_tensor(out=scl[:], in0=gb_t[:, l:l + 1],
                                        in1=tmp[:], op=OP.mult)
                nc.vector.tensor_tensor(out=tmp[:], in0=mean[:], in1=scl[:],
                                        op=OP.mult)
                nc.vector.tensor_tensor(out=sht[:], in0=gb_t[:, L + l:L + l + 1],
                                        in1=tmp[:], op=OP.subtract)

                # ---------------- normalize (+ relu except last layer)
                nc.scalar.activation(out=h_t[:], in_=h_t[:],
                                     func=AF.Relu if l < L - 1 else AF.Identity,
                                     bias=sht[:], scale=scl[:])

                # ---------------- next-layer tables + allgather
                if l < L - 1:
                    for t in (0, 1):
                        tb = gbf_t[t]  # reuse as bf16 staging (dense done)
                        nc.vector.tensor_tensor(out=tb[:], in0=h_t[:],
                                                in1=dinv_t[t][:], op=OP.mult)
                        for w_ in range(NW):
                            wd = min(128, S - w_ * 128)
                            sl = slice(w_ * 128, w_ * 128 + wd)
                            tp = psT.tile([128, 128], bf16, space="PSUM",
                                          tag="tp")
                            nc.tensor.transpose(out=tp[:wd, :], in_=tb[:, sl],
                                                identity=ident_t[:])
                            tsb = scp.tile([128, 128], bf16, tag="tsb", bufs=3)
                            nc.vector.tensor_copy(out=tsb[:wd, :],
                                                  in_=tp[:wd, :])
                            nc.sync.dma_start(ag_in[(l, t)][sl], tsb[:wd, :])
                        nc.gpsimd.collective_compute(
                            "AllGather", OP.bypass, replica_groups=rg,
                            ins=[ag_in[(l, t)].opt()],
                            outs=[ag_out[(l, t)].opt()])

            # ---------------- heads
            def l2norm(dst_t_, x_t_, fw):
                sqb = scp.tile([128, 512], bf16, tag="sqb")
                nc.scalar.activation(out=sqb[:, :fw], in_=x_t_[:, :fw],
                                     func=AF.Square)
                nsq = psB.tile([128, 512], f32, space="PSUM", tag="dense")
                nc.tensor.matmul(out=nsq[:, :fw], lhsT=ones_t[:],
                                 rhs=sqb[:, :fw], start=True, stop=True)
                nrm = scp.tile([128, 512], f32, tag="nrm")
                nc.scalar.activation(out=nrm[:, :fw], in_=nsq[:, :fw],
                                     func=AF.Sqrt)
                nc.vector.tensor_scalar(out=nrm[:, :fw], in0=nrm[:, :fw],
                                        scalar1=EPS_NORM, scalar2=None,
                                        op0=OP.max)
                nc.vector.reciprocal(out=nrm[:, :fw], in_=nrm[:, :fw])
                nc.vector.tensor_tensor(out=dst_t_[:, :fw], in0=x_t_[:, :fw],
                                        in1=nrm[:, :fw], op=OP.mult)

            for ft in range(NF):
                fw = min(512, S - ft * 512)
                sl = slice(ft * 512, ft * 512 + fw)
                hbf = scp.tile([128, 512], bf16, tag="hbf", bufs=2)
                nc.vector.tensor_copy(out=hbf[:, :fw], in_=h_t[:, sl])

                # e1 branch
                e1p = psB.tile([128, 512], f32, space="PSUM", tag="dense")
                nc.tensor.matmul(out=e1p[:, :fw], lhsT=wh_t[:, 0, :],
                                 rhs=hbf[:, :fw], start=True, stop=True)
                e1s = scp.tile([128, 512], f32, tag="e1s")
                nc.scalar.activation(out=e1s[:, :fw], in_=e1p[:, :fw],
                                     func=AF.Tanh, bias=hb_t[:, 0:1])
                nc.sync.dma_start(outs[0][:, sl], e1s[:, :fw])
                e1b = scp.tile([128, 512], bf16, tag="e1b")
                nc.vector.tensor_copy(out=e1b[:, :fw], in_=e1s[:, :fw])
                r1p = psB.tile([128, 512], f32, space="PSUM", tag="dense")
                nc.tensor.matmul(out=r1p[:, :fw], lhsT=wh_t[:, 2, :],
                                 rhs=e1b[:, :fw], start=True, stop=True)
                r1b = scp.tile([128, 512], bf16, tag="r1b")
                nc.scalar.activation(out=r1b[:, :fw], in_=r1p[:, :fw],
                                     func=AF.Relu, bias=hb_t[:, 2:3])
                z1p = psB.tile([128, 512], f32, space="PSUM", tag="dense")
                nc.tensor.matmul(out=z1p[:, :fw], lhsT=wh_t[:, 3, :],
                                 rhs=r1b[:, :fw], start=True, stop=True)
                z1s = scp.tile([128, 512], f32, tag="z1s")
                nc.scalar.activation(out=z1s[:, :fw], in_=z1p[:, :fw],
                                     func=AF.Identity, bias=hb_t[:, 3:4])
                p1s = scp.tile([128, 512], f32, tag="p1s")
                l2norm(p1s, z1s, fw)
                nc.sync.dma_start(outs[2][:, sl], p1s[:, :fw])

                # e2 branch
                e2p = psB.tile([128, 512], f32, space="PSUM", tag="dense")
                nc.tensor.matmul(out=e2p[:, :fw], lhsT=wh_t[:, 1, :],
                                 rhs=hbf[:, :fw], start=True, stop=True)
                t2s = scp.tile([128, 512], f32, tag="t2s")
                nc.scalar.activation(out=t2s[:, :fw], in_=e2p[:, :fw],
                                     func=AF.Tanh, bias=hb_t[:, 1:2])
                e2s = scp.tile([128, 512], f32, tag="e2s")
                l2norm(e2s, t2s, fw)
                nc.sync.dma_start(outs[1][:, sl], e2s[:, :fw])
                e2b = scp.tile([128, 512], bf16, tag="e2b")
                nc.vector.tensor_copy(out=e2b[:, :fw], in_=e2s[:, :fw])
                r2p = psB.tile([128, 512], f32, space="PSUM", tag="dense")
                nc.tensor.matmul(out=r2p[:, :fw], lhsT=wh_t[:, 4, :],
                                 rhs=e2b[:, :fw], start=True, stop=True)
                r2b = scp.tile([128, 512], bf16, tag="r2b")
                nc.scalar.activation(out=r2b[:, :fw], in_=r2p[:, :fw],
                                     func=AF.Relu, bias=hb_t[:, 4:5])
                z2p = psB.tile([128, 512], f32, space="PSUM", tag="dense")
                nc.tensor.matmul(out=z2p[:, :fw], lhsT=wh_t[:, 5, :],
                                 rhs=r2b[:, :fw], start=True, stop=True)
                z2s = scp.tile([128, 512], f32, tag="z2s")
                nc.scalar.activation(out=z2s[:, :fw], in_=z2p[:, :fw],
                                     func=AF.Identity, bias=hb_t[:, 5:6])
                p2s = scp.tile([128, 512], f32, tag="p2s")
                l2norm(p2s, z2s, fw)
                nc.sync.dma_start(outs[3][:, sl], p2s[:, :fw])

    nc.compile()
    return nc


# ---------------------------------------------------------------- entry point

def _run(inputs, trace=False, trace_kwargs=None):
    x = np.asarray(inputs["x"], np.float32)
    N = x.shape[0]
    assert N % NCORES == 0
    S = N // NCORES

    dinv0, sch0, idx0, rel0, stot0, nch0 = _prep_type(inputs["edge_index0"], N, S)
    dinv1, sch1, idx1, rel1, stot1, nch1 = _prep_type(inputs["edge_index1"], N, S)

    nc = _build(N, S, sch0, stot0, nch0, sch1, stot1, nch1)

    tab0 = (x * dinv0[:, None]).astype(np.float16)
    tab1 = (x * dinv1[:, None]).astype(np.float16)

    W0 = np.asarray(inputs["W0"], np.float32)
    W1 = np.asarray(inputs["W1"], np.float32)
    wd = np.zeros((L * 2 * 128, D), np.float32)
    for l in range(L):
        wd[(l * 2) * 128:(l * 2 + 1) * 128] = W0[l]
        wd[(l * 2 + 1) * 128:(l * 2 + 2) * 128] = W1[l]
    gb = np.stack([np.asarray(inputs["gamma"], np.float32).T,
                   np.asarray(inputs["beta"], np.float32).T], 0)
    gb = np.concatenate([gb[0], gb[1]], axis=1)  # [128, 2L]
    wh = np.concatenate([np.asarray(inputs[k], np.float32) for k in
                         ("emb1_W", "emb2_W", "ph1_Wa", "ph1_Wb",
                          "ph2_Wa", "ph2_Wb")], 0)
    hb = np.stack([np.asarray(inputs[k], np.float32) for k in
                   ("emb1_b", "emb2_b", "ph1_ba", "ph1_bb",
                    "ph2_ba", "ph2_bb")], 1)

    iota = np.broadcast_to(np.arange(128, dtype=np.float32),
                           (128, 128)).astype(np.float16)
    ident = np.eye(128, dtype=np.float16)
    ones = np.ones((128, 128), np.float16)

    in_maps = []
    for c in range(NCORES):
        sl = slice(c * S, (c + 1) * S)
        in_maps.append({
            "tab0_in": tab0, "tab1_in": tab1,
            "xT_in": np.ascontiguousarray(x[sl].T),
            "dinv0_in": np.ascontiguousarray(
                np.broadcast_to(dinv0[sl], (128, S))).astype(np.float16),
            "dinv1_in": np.ascontiguousarray(
                np.broadcast_to(dinv1[sl], (128, S))).astype(np.float16),
            "idx0_in": idx0[c], "idx1_in": idx1[c],
            "rel0_in": rel0[c], "rel1_in": rel1[c],
            "wd_in": wd.astype(np.float16),
            "gb_in": gb, "wh_in": wh.astype(np.float16), "hb_in": hb,
            "iota_in": iota, "ident_in": ident, "ones_in": ones,
        })

    res = run_bass_kernel_spmd(nc, in_maps, list(range(NCORES)),
                               trace=trace, **(trace_kwargs or {}))

    full = {}
    for name in ("e1_o", "e2_o", "p1_o", "p2_o"):
        full[name] = np.concatenate(
            [res.results[c][name].T for c in range(NCORES)], axis=0)
    return (full["e1_o"], full["e2_o"], full["p1_o"], full["p2_o"]), res


def kernel(**inputs):
    out, _ = _run(inputs)
    return out


# revision 18
# speedup vs baseline: 5.3783x; 5.3783x over previous
"""Trainium2 Bass kernel for nn_ClusterGCN (3-layer 2-edge-type GCN + heads).

Strategy (8 NeuronCores, node-parallel):
  - Nodes sharded contiguously: core c owns rows [c*S, (c+1)*S), S = N/8.
  - Per layer, each core holds a replicated node-major bf16 table of
    h_tilde_t = dinv_t * h (one per edge type) in DRAM; edges are sharded by
    dst. Messages h_tilde[src] are fetched with GPSIMD dma_gather (int16
    indices, lo/hi base split for N > 32768) and scatter-added into a
    feature-major accumulator via one-hot matmuls on the PE
    (out[feat, dst_slot] += msg[edge, feat]^T @ onehot[edge, dst_slot]).
  - g_t = dinv_t * (scatter + dinv_t * h) adds the self-loop, then
    h' = BN(g0 @ W0 + g1 @ W1) with batch stats AllReduced across cores.
  - Tables for the next layer are rebuilt feature-major, PE-transposed to
    node-major, and AllGathered.  Heads (tanh/relu/l2norm MLPs) run
    node-sharded at the end.
"""
import math
import numpy as np
import ml_dtypes

import concourse.bacc as bacc
import concourse.bass as bass
import concourse.mybir as mybir
import concourse.tile as tile
from concourse.library_config import mlp as mlp_lib
from concourse.bass_utils import run_bass_kernel_spmd

NCORES = 8
D = 128
L = 3
EPS_BN = 1e-5
EPS_NORM = 1e-12
IDX_LIMIT = 32768
PIECE = 1024          # gather slots per dma_gather instruction
SENT_DST = 320.0      # sentinel dst slot (bf16-exact, >= 128)

f32 = mybir.dt.float32
bf16 = mybir.dt.float16  # (fp16 everywhere: 8x finer mantissa than bf16, same HW rates)
i16 = mybir.dt.int16
AF = mybir.ActivationFunctionType
OP = mybir.AluOpType


# ---------------------------------------------------------------- host prep

def _prep_type(edge_index, N, S):
    """Per edge type: degrees + per-core common-shape gather/scatter schedule."""
    src = np.asarray(edge_index[0], np.int64)
    dst = np.asarray(edge_index[1], np.int64)
    deg = np.bincount(dst, minlength=N).astype(np.float32) + 1.0
    dinv = (1.0 / np.sqrt(deg)).astype(np.float32)

    NW = (S + 127) // 128
    HI_BASE = N - IDX_LIMIT  # hi-region table base; rows [HI_BASE, N)
    # src in [0, IDX_LIMIT) reachable from region 0; [HI_BASE, N) from region 1.
    # srcs in the overlap [max(HI_BASE,0), IDX_LIMIT) are flexible - used to
    # round region-0 groups up to full chunks and minimize sentinel padding.
    cores = []
    for c in range(NCORES):
        m = (dst >= c * S) & (dst < (c + 1) * S)
        s_c = src[m]
        dl = dst[m] - c * S
        w = dl // 128
        order = np.lexsort((s_c, dl, w))
        cores.append((s_c[order], dl[order], w[order]))

    if N > IDX_LIMIT:
        # chunk counts per (region, window): region 0 gets
        # K0 = max_c ceil(must_lo/128); flexible srcs fill region 0 up to
        # K0*128, remainder goes to region 1.
        K = np.zeros((2, NW), np.int64)
        must_lo = []
        for (s_c, dl, w) in cores:
            cnt_lo = np.bincount(w[s_c < HI_BASE], minlength=NW)
            must_lo.append(cnt_lo)
            K[0] = np.maximum(K[0], (cnt_lo + 127) // 128)
        K[0] = np.maximum(K[0], 1)
        core_reg = []
        for ci, (s_c, dl, w) in enumerate(cores):
            reg = (s_c >= IDX_LIMIT).astype(np.int64)
            for w_ in range(NW):
                cap = K[0][w_] * 128
                flex = np.flatnonzero((w == w_) & (s_c >= HI_BASE) & (s_c < IDX_LIMIT))
                take = min(max(cap - int(must_lo[ci][w_]), 0), len(flex))
                reg[flex[:take]] = 0
                reg[flex[take:]] = 1
            cnt_hi = np.bincount(w[reg == 1], minlength=NW)
            K[1] = np.maximum(K[1], (cnt_hi + 127) // 128)
            core_reg.append(reg)
        K[1] = np.maximum(K[1], 1)
        cores = [(s_c, dl, w, core_reg[ci]) for ci, (s_c, dl, w) in enumerate(cores)]
    else:
        K = np.zeros((2, NW), np.int64)
        for ci, (s_c, dl, w) in enumerate(cores):
            cnt = np.bincount(w, minlength=NW)
            K[0] = np.maximum(K[0], (cnt + 127) // 128)
        K[0] = np.maximum(K[0], 1)
        cores = [(s_c, dl, w, np.zeros(len(s_c), np.int64)) for (s_c, dl, w) in cores]

    schedule = []  # (region, window, nchunks) in slot order
    for r in (0, 1):
        for w_ in range(NW):
            if K[r][w_] > 0:
                schedule.append((r, int(w_), int(K[r][w_])))
    nchunks = sum(k for _, _, k in schedule)
    stot = nchunks * 128

    idx_all = np.zeros((NCORES, max(stot, 128)), np.int64)
    rel_all = np.full((NCORES, max(nchunks, 1) * 128), SENT_DST, np.float64)
    for ci, (s_c, dl, w, reg) in enumerate(cores):
        pos = 0
        for (r, w_, k) in schedule:
            m = (reg == r) & (w == w_)
            n = int(m.sum())
            sv = s_c[m]
            idx_all[ci, pos:pos + n] = sv if r == 0 else sv - (N - IDX_LIMIT)
            rel_all[ci, pos:pos + n] = dl[m] - w_ * 128
            pos += k * 128

    # wrapped int16 index layout: idxs[p, s] = idx[s*16 + p%16]
    cols = max(stot // 16, 1)
    idx_w = np.zeros((NCORES, 128, cols), np.int16)
    if stot:
        a = idx_all[:, :stot].reshape(NCORES, cols, 16)  # [c, s, j]
        for p in range(128):
            idx_w[:, p, :] = a[:, :, p % 16]
    # dst-slot tile: rel[p, chunk] = slot of edge chunk*128+p
    rel_t = np.ascontiguousarray(
        rel_all[:, :nchunks * 128].reshape(NCORES, nchunks, 128).transpose(0, 2, 1)
    ).astype(np.float32)

    return dinv, schedule, idx_w, rel_t, stot, nchunks


def _pieces(schedule):
    """Split slot range into gather pieces that do not cross the lo/hi boundary.
    Returns list of (slot_start, slot_count, region)."""
    out = []
    for r in (0, 1):
        lo = sum(k * 128 for (rr, _, k) in schedule if rr < r)
        n = sum(k * 128 for (rr, _, k) in schedule if rr == r)
        p = lo
        while p < lo + n:
            c = min(PIECE, lo + n - p)
            out.append((p, c, r))
            p += c
    return out


# ---------------------------------------------------------------- device build

def _build(N, S, sch0, stot0, nch0, sch1, stot1, nch1):
    NW = (S + 127) // 128
    NF = (S + 511) // 512  # 512-wide node tiles
    nc = bacc.Bacc("TRN2", target_bir_lowering=False, debug=False,
                   num_devices=NCORES)

    def din(name, shape, dt):
        return nc.dram_tensor(name, shape, dt, kind="ExternalInput")

    tab_in = [din("tab0_in", [N, D], bf16), din("tab1_in", [N, D], bf16)]
    xT_in = din("xT_in", [128, S], f32)
    dinv_in = [din("dinv0_in", [128, S], bf16), din("dinv1_in", [128, S], bf16)]
    idx_in = [din("idx0_in", [128, max(stot0 // 16, 1)], i16),
              din("idx1_in", [128, max(stot1 // 16, 1)], i16)]
    rel_in = [din("rel0_in", [128, max(nch0, 1)], f32),
              din("rel1_in", [128, max(nch1, 1)], f32)]
    wd_in = din("wd_in", [L * 2 * 128, D], bf16)
    gb_in = din("gb_in", [128, 2 * L], f32)
    wh_in = din("wh_in", [6 * 128, D], bf16)
    hb_in = din("hb_in", [128, 6], f32)
    iota_in = din("iota_in", [128, 128], bf16)
    ident_in = din("ident_in", [128, 128], bf16)
    ones_in = din("ones_in", [128, 128], bf16)

    outs = [nc.dram_tensor(n, [128, S], f32, kind="ExternalOutput")
            for n in ("e1_o", "e2_o", "p1_o", "p2_o")]

    with tile.TileContext(nc) as tc:
        with (
            tc.tile_pool(name="const", bufs=1) as const,
            tc.tile_pool(name="g", bufs=1) as gpool,
            tc.tile_pool(name="msg", bufs=3) as msgp,
            tc.tile_pool(name="oh", bufs=6) as ohp,
            tc.tile_pool(name="scr", bufs=1) as scp,
            tc.tile_pool(name="psA", bufs=4, space="PSUM") as psA,
            tc.tile_pool(name="psT", bufs=2, space="PSUM") as psT,
            tc.tile_pool(name="psB", bufs=2, space="PSUM") as psB,
            tc.tile_pool(name="dram", bufs=1, space="DRAM") as dram,
        ):
            nc.gpsimd.load_library(mlp_lib)

            # ---- persistent SBUF tiles
            iota_t = const.tile([128, 128], bf16)
            ident_t = const.tile([128, 128], bf16)
            ones_t = const.tile([128, 128], bf16)
            dinv_t = [const.tile([128, S], bf16, tag=f"dinv{t}", name=f"dinv{t}") for t in (0, 1)]
            idx_t = [const.tile([128, max(stot0 // 16, 1)], i16, tag="idx0", name="idx0"),
                     const.tile([128, max(stot1 // 16, 1)], i16, tag="idx1", name="idx1")]
            rel_t = [const.tile([128, max(nch0, 1)], f32, tag="rel0", name="rel0"),
                     const.tile([128, max(nch1, 1)], f32, tag="rel1", name="rel1")]
            wd_t = const.tile([128, L * 2, D], bf16)     # dense weights
            wh_t = const.tile([128, 6, D], bf16)         # head weights
            gb_t = const.tile([128, 2 * L], f32)
            hb_t = const.tile([128, 6], f32)

            h_t = gpool.tile([128, S], f32, tag="h")
            h16_t = gpool.tile([128, S], bf16, tag="h16")
            gbf_t = [gpool.tile([128, S], bf16, tag=f"gbf{t}", name=f"gbf{t}") for t in (0, 1)]

            nc.sync.dma_start(iota_t[:], iota_in[:])
            nc.sync.dma_start(ident_t[:], ident_in[:])
            nc.sync.dma_start(ones_t[:], ones_in[:])
            for t in (0, 1):
                nc.sync.dma_start(dinv_t[t][:], dinv_in[t][:])
                nc.sync.dma_start(idx_t[t][:], idx_in[t][:])
                nc.sync.dma_start(rel_t[t][:], rel_in[t][:])
            nc.sync.dma_start(
                wd_t[:], wd_in[:].rearrange("(k p) d -> p k d", p=128))
            nc.sync.dma_start(
                wh_t[:], wh_in[:].rearrange("(k p) d -> p k d", p=128))
            nc.sync.dma_start(gb_t[:], gb_in[:])
            nc.sync.dma_start(hb_t[:], hb_in[:])
            nc.sync.dma_start(h_t[:], xT_in[:])
            nc.vector.tensor_copy(out=h16_t[:], in_=h_t[:])

            # ---- internal DRAM for collectives
            ag_in = {}
            ag_out = {}
            for l in (0, 1):
                for t in (0, 1):
                    ag_in[(l, t)] = dram.tile([S, D], bf16, tag=f"agi{l}{t}", name=f"agi{l}{t}")
                    ag_out[(l, t)] = dram.tile([N, D], bf16,
                                               addr_space="Shared",
                                               tag=f"ago{l}{t}",
                                               name=f"ago{l}{t}")
            st_in = [dram.tile([128, 2], f32, tag=f"sti{l}", name=f"sti{l}") for l in range(L)]
            st_out = [dram.tile([128, 2], f32, addr_space="Shared",
                                tag=f"sto{l}", name=f"sto{l}") for l in range(L)]

            schs = (sch0, sch1)
            rg = [list(range(NCORES))]

            for l in range(L):
                # ---------------- scatter phase (both edge types)
                for t in (0, 1):
                    # g init: self-loop inner term  g = dinv * h  (fp16, 4x DVE)
                    nc.vector.tensor_tensor(out=gbf_t[t][:], in0=h16_t[:],
                                            in1=dinv_t[t][:], op=OP.mult)
                    if l == 0:
                        tab_lo = tab_in[t][:]
                        tab_hi = tab_in[t][N - IDX_LIMIT:] if N > IDX_LIMIT else None
                    else:
                        tab_lo = ag_out[(l - 1, t)][:]
                        tab_hi = ag_out[(l - 1, t)][N - IDX_LIMIT:] \
                            if N > IDX_LIMIT else None

                    sch = schs[t]
                    # chunk meta: (region, window, win_first, win_last,
                    #              group_first, group_last); groups = 4 windows
                    # of one region sharing a [128,512] PSUM bank.
                    chunk_meta = []
                    for si, (r, w_, k) in enumerate(sch):
                        gf = (w_ % 4 == 0) or si == 0 or sch[si - 1][0] != r
                        gl = (w_ % 4 == 3) or si == len(sch) - 1 \
                            or sch[si + 1][0] != r
                        for j in range(k):
                            chunk_meta.append(
                                (r, w_, j == 0, j == k - 1,
                                 gf and j == 0, gl and j == k - 1))

                    pieces = _pieces(sch)
                    acc = None
                    for (p0, cnt, r) in pieces:
                        msg = msgp.tile([128, PIECE // 128, 128], bf16,
                                        tag="msg")
                        src_ap = tab_lo if r == 0 else tab_hi
                        nc.gpsimd.dma_gather(
                            msg[:, :cnt // 128, :], src_ap,
                            idx_t[t][:, p0 // 16:(p0 + cnt) // 16],
                            num_idxs=cnt, num_idxs_reg=cnt, elem_size=D,
                        )
                        for ci in range(cnt // 128):
                            gc = p0 // 128 + ci
                            (cr, w_, first, last, gfirst, glast) = chunk_meta[gc]
                            oh = ohp.tile([128, 128], bf16, tag="oh")
                            nc.vector.tensor_scalar(
                                out=oh[:], in0=iota_t[:],
                                scalar1=rel_t[t][:, gc:gc + 1], scalar2=None,
                                op0=OP.is_equal,
                            )
                            if gfirst:
                                acc = psA.tile([128, 512], f32, space="PSUM",
                                               tag="sc")
                            ws = (w_ % 4) * 128
                            nc.tensor.matmul(out=acc[:, ws:ws + 128],
                                             lhsT=msg[:, ci, :],
                                             rhs=oh[:], start=first, stop=last)
                            if glast:
                                base = (w_ // 4) * 512
                                wd = min(512, S - base)
                                nc.vector.tensor_tensor(
                                    out=gbf_t[t][:, base:base + wd],
                                    in0=gbf_t[t][:, base:base + wd],
                                    in1=acc[:, :wd], op=OP.add)
                    # final dst-side scale per 512-tile (lets dense overlap)
                    for ft in range(NF):
                        fsl = slice(ft * 512, min(ft * 512 + 512, S))
                        nc.vector.tensor_tensor(out=gbf_t[t][:, fsl],
                                                in0=gbf_t[t][:, fsl],
                                                in1=dinv_t[t][:, fsl],
                                                op=OP.mult)

                # ---------------- dense + stats partials
                sum_p = scp.tile([128, NF], f32, tag="sump")
                ssq_p = scp.tile([128, NF], f32, tag="ssqp")
                for ft in range(NF):
                    fw = min(512, S - ft * 512)
                    sl = slice(ft * 512, ft * 512 + fw)
                    dp = psB.tile([128, 512], f32, space="PSUM", tag="dense")
                    nc.tensor.matmul(out=dp[:, :fw], lhsT=wd_t[:, l * 2, :],
                                     rhs=gbf_t[0][:, sl], start=True, stop=False)
                    nc.tensor.matmul(out=dp[:, :fw], lhsT=wd_t[:, l * 2 + 1, :],
                                     rhs=gbf_t[1][:, sl], start=False, stop=True)
                    nc.vector.tensor_reduce(out=sum_p[:, ft:ft + 1],
                                            in_=dp[:, :fw],
                                            axis=mybir.AxisListType.X,
                                            op=OP.add)
                    sq = scp.tile([128, 512], f32, tag="sq", bufs=2)
                    nc.scalar.activation(out=sq[:, :fw], in_=dp[:, :fw],
                                         func=AF.Square,
                                         accum_out=ssq_p[:, ft:ft + 1])
                    nc.scalar.activation(out=h_t[:, sl], in_=dp[:, :fw],
                                         func=AF.Copy)

                # ---------------- BN stats allreduce
                st = scp.tile([128, 2], f32, tag="st")
                nc.vector.tensor_reduce(out=st[:, 0:1], in_=sum_p[:],
                                        axis=mybir.AxisListType.X, op=OP.add)
                nc.vector.tensor_reduce(out=st[:, 1:2], in_=ssq_p[:],
                                        axis=mybir.AxisListType.X, op=OP.add)
                nc.sync.dma_start(st_in[l][:], st[:])
                nc.gpsimd.collective_compute(
                    "AllReduce", OP.add, replica_groups=rg,
                    ins=[st_in[l].opt()], outs=[st_out[l].opt()])
                sta = scp.tile([128, 2], f32, tag="sta")
                nc.sync.dma_start(sta[:], st_out[l][:])

                mean = scp.tile([128, 1], f32, tag="mean")
                var = scp.tile([128, 1], f32, tag="var")
                scl = scp.tile([128, 1], f32, tag="scl")
                sht = scp.tile([128, 1], f32, tag="sht")
                tmp = scp.tile([128, 1], f32, tag="tmp1")
                inv_n = 1.0 / float(N)
                nc.vector.tensor_scalar(out=mean[:], in0=sta[:, 0:1],
                                        scalar1=inv_n, scalar2=None, op0=OP.mult)
                nc.vector.tensor_scalar(out=var[:], in0=sta[:, 1:2],
                                        scalar1=inv_n, scalar2=None, op0=OP.mult)
                nc.vector.tensor_tensor(out=tmp[:], in0=mean[:], in1=mean[:],
                                        op=OP.mult)
                nc.vector.tensor_tensor(out=var[:], in0=var[:], in1=tmp[:],
                                        op=OP.subtract)
                # scl = gamma / sqrt(var + eps); sht = beta - mean*scl
                nc.vector.tensor_scalar(out=var[:], in0=var[:], scalar1=EPS_BN,
                                        scalar2=None, op0=OP.add)
                nc.scalar.activation(out=tmp[:], in_=var[:], func=AF.Sqrt)
                nc.vector.reciprocal(out=tmp[:], in_=tmp[:])
                nc.vector.tensor_tensor(out=scl[:], in0=gb_t[:, l:l + 1],
                                        in1=tmp[:], op=OP.mult)
                nc.vector.tensor_tensor(out=tmp[:], in0=mean[:], in1=scl[:],
                                        op=OP.mult)
                nc.vector.tensor_tensor(out=sht[:], in0=gb_t[:, L + l:L + l + 1],
                                        in1=tmp[:], op=OP.subtract)

                # ---------------- normalize (+ relu except last layer)
                nc.scalar.activation(out=h16_t[:], in_=h_t[:],
                                     func=AF.Relu if l < L - 1 else AF.Identity,
                                     bias=sht[:], scale=scl[:])

                # ---------------- next-layer tables + allgather
                if l < L - 1:
                    for t in (0, 1):
                        tb = gbf_t[t]  # reuse as bf16 staging (dense done)
                        nc.vector.tensor_tensor(out=tb[:], in0=h16_t[:],
                                                in1=dinv_t[t][:], op=OP.mult)
                        for w_ in range(NW):
                            wd = min(128, S - w_ * 128)
                            sl = slice(w_ * 128, w_ * 128 + wd)
                            tp = psT.tile([128, 128], bf16, space="PSUM",
                                          tag="tp")
                            nc.tensor.transpose(out=tp[:wd, :], in_=tb[:, sl],
                                                identity=ident_t[:])
                            tsb = scp.tile([128, 128], bf16, tag="tsb", bufs=3)
                            nc.scalar.activation(out=tsb[:wd, :],
                                                 in_=tp[:wd, :], func=AF.Copy)
                            nc.sync.dma_start(ag_in[(l, t)][sl], tsb[:wd, :])
                        nc.gpsimd.collective_compute(
                            "AllGather", OP.bypass, replica_groups=rg,
                            ins=[ag_in[(l, t)].opt()],
                            outs=[ag_out[(l, t)].opt()])

            # ---------------- heads
            def l2norm(dst_t_, x_t_, fw):
                sqb = scp.tile([128, 512], bf16, tag="sqb")
                nc.scalar.activation(out=sqb[:, :fw], in_=x_t_[:, :fw],
                                     func=AF.Square)
                nsq = psB.tile([128, 512], f32, space="PSUM", tag="dense")
                nc.tensor.matmul(out=nsq[:, :fw], lhsT=ones_t[:],
                                 rhs=sqb[:, :fw], start=True, stop=True)
                nrm = scp.tile([128, 512], f32, tag="nrm")
                nc.scalar.activation(out=nrm[:, :fw], in_=nsq[:, :fw],
                                     func=AF.Sqrt)
                nc.vector.tensor_scalar(out=nrm[:, :fw], in0=nrm[:, :fw],
                                        scalar1=EPS_NORM, scalar2=None,
                                        op0=OP.max)
                nc.vector.reciprocal(out=nrm[:, :fw], in_=nrm[:, :fw])
                nc.vector.tensor_tensor(out=dst_t_[:, :fw], in0=x_t_[:, :fw],
                                        in1=nrm[:, :fw], op=OP.mult)

            for ft in range(NF):
                fw = min(512, S - ft * 512)
                sl = slice(ft * 512, ft * 512 + fw)
                # e1 branch
                e1p = psB.tile([128, 512], f32, space="PSUM", tag="dense")
                nc.tensor.matmul(out=e1p[:, :fw], lhsT=wh_t[:, 0, :],
                                 rhs=h16_t[:, sl], start=True, stop=True)
                e1s = scp.tile([128, 512], f32, tag="e1s")
                nc.scalar.activation(out=e1s[:, :fw], in_=e1p[:, :fw],
                                     func=AF.Tanh, bias=hb_t[:, 0:1])
                nc.sync.dma_start(outs[0][:, sl], e1s[:, :fw])
                e1b = scp.tile([128, 512], bf16, tag="e1b")
                nc.vector.tensor_copy(out=e1b[:, :fw], in_=e1s[:, :fw])
                r1p = psB.tile([128, 512], f32, space="PSUM", tag="dense")
                nc.tensor.matmul(out=r1p[:, :fw], lhsT=wh_t[:, 2, :],
                                 rhs=e1b[:, :fw], start=True, stop=True)
                r1b = scp.tile([128, 512], bf16, tag="r1b")
                nc.scalar.activation(out=r1b[:, :fw], in_=r1p[:, :fw],
                                     func=AF.Relu, bias=hb_t[:, 2:3])
                z1p = psB.tile([128, 512], f32, space="PSUM", tag="dense")
                nc.tensor.matmul(out=z1p[:, :fw], lhsT=wh_t[:, 3, :],
                                 rhs=r1b[:, :fw], start=True, stop=True)
                z1s = scp.tile([128, 512], f32, tag="z1s")
                nc.scalar.activation(out=z1s[:, :fw], in_=z1p[:, :fw],
                                     func=AF.Identity, bias=hb_t[:, 3:4])
                p1s = scp.tile([128, 512], f32, tag="p1s")
                l2norm(p1s, z1s, fw)
                nc.sync.dma_start(outs[2][:, sl], p1s[:, :fw])

                # e2 branch
                e2p = psB.tile([128, 512], f32, space="PSUM", tag="dense")
                nc.tensor.matmul(out=e2p[:, :fw], lhsT=wh_t[:, 1, :],
                                 rhs=h16_t[:, sl], start=True, stop=True)
                t2s = scp.tile([128, 512], f32, tag="t2s")
                nc.scalar.activation(out=t2s[:, :fw], in_=e2p[:, :fw],
                                     func=AF.Tanh, bias=hb_t[:, 1:2])
                e2s = scp.tile([128, 512], f32, tag="e2s")
                l2norm(e2s, t2s, fw)
                nc.sync.dma_start(outs[1][:, sl], e2s[:, :fw])
                e2b = scp.tile([128, 512], bf16, tag="e2b")
                nc.vector.tensor_copy(out=e2b[:, :fw], in_=e2s[:, :fw])
                r2p = psB.tile([128, 512], f32, space="PSUM", tag="dense")
                nc.tensor.matmul(out=r2p[:, :fw], lhsT=wh_t[:, 4, :],
                                 rhs=e2b[:, :fw], start=True, stop=True)
                r2b = scp.tile([128, 512], bf16, tag="r2b")
                nc.scalar.activation(out=r2b[:, :fw], in_=r2p[:, :fw],
                                     func=AF.Relu, bias=hb_t[:, 4:5])
                z2p = psB.tile([128, 512], f32, space="PSUM", tag="dense")
                nc.tensor.matmul(out=z2p[:, :fw], lhsT=wh_t[:, 5, :],
                                 rhs=r2b[:, :fw], start=True, stop=True)
                z2s = scp.tile([128, 512], f32, tag="z2s")
                nc.scalar.activation(out=z2s[:, :fw], in_=z2p[:, :fw],
                                     func=AF.Identity, bias=hb_t[:, 5:6])
                p2s = scp.tile([128, 512], f32, tag="p2s")
                l2norm(p2s, z2s, fw)
                nc.sync.dma_start(outs[3][:, sl], p2s[:, :fw])

    nc.compile()
    return nc


# ---------------------------------------------------------------- entry point

def _run(inputs, trace=False, trace_kwargs=None, nc_out=None):
    x = np.asarray(inputs["x"], np.float32)
    N = x.shape[0]
    assert N % NCORES == 0
    S = N // NCORES

    dinv0, sch0, idx0, rel0, stot0, nch0 = _prep_type(inputs["edge_index0"], N, S)
    dinv1, sch1, idx1, rel1, stot1, nch1 = _prep_type(inputs["edge_index1"], N, S)

    nc = _build(N, S, sch0, stot0, nch0, sch1, stot1, nch1)
    if nc_out is not None:
        nc_out.append(nc)

    tab0 = (x * dinv0[:, None]).astype(np.float16)
    tab1 = (x * dinv1[:, None]).astype(np.float16)

    W0 = np.asarray(inputs["W0"], np.float32)
    W1 = np.asarray(inputs["W1"], np.float32)
    wd = np.zeros((L * 2 * 128, D), np.float32)
    for l in range(L):
        wd[(l * 2) * 128:(l * 2 + 1) * 128] = W0[l]
        wd[(l * 2 + 1) * 128:(l * 2 + 2) * 128] = W1[l]
    gb = np.stack([np.asarray(inputs["gamma"], np.float32).T,
                   np.asarray(inputs["beta"], np.float32).T], 0)
    gb = np.concatenate([gb[0], gb[1]], axis=1)  # [128, 2L]
    wh = np.concatenate([np.asarray(inputs[k], np.float32) for k in
                         ("emb1_W", "emb2_W", "ph1_Wa", "ph1_Wb",
                          "ph2_Wa", "ph2_Wb")], 0)
    hb = np.stack([np.asarray(inputs[k], np.float32) for k in
                   ("emb1_b", "emb2_b", "ph1_ba", "ph1_bb",
                    "ph2_ba", "ph2_bb")], 1)

    iota = np.broadcast_to(np.arange(128, dtype=np.float32),
                           (128, 128)).astype(np.float16)
    ident = np.eye(128, dtype=np.float16)
    ones = np.ones((128, 128), np.float16)

    in_maps = []
    for c in range(NCORES):
        sl = slice(c * S, (c + 1) * S)
        in_maps.append({
            "tab0_in": tab0, "tab1_in": tab1,
            "xT_in": np.ascontiguousarray(x[sl].T),
            "dinv0_in": np.ascontiguousarray(
                np.broadcast_to(dinv0[sl], (128, S))).astype(np.float16),
            "dinv1_in": np.ascontiguousarray(
                np.broadcast_to(dinv1[sl], (128, S))).astype(np.float16),
            "idx0_in": idx0[c], "idx1_in": idx1[c],
            "rel0_in": rel0[c], "rel1_in": rel1[c],
            "wd_in": wd.astype(np.float16),
            "gb_in": gb, "wh_in": wh.astype(np.float16), "hb_in": hb,
            "iota_in": iota, "ident_in": ident, "ones_in": ones,
        })

    res = run_bass_kernel_spmd(nc, in_maps, list(range(NCORES)),
                               trace=trace, **(trace_kwargs or {}))

    full = {}
    for name in ("e1_o", "e2_o", "p1_o", "p2_o"):
        full[name] = np.concatenate(
            [res.results[c][name].T for c in range(NCORES)], axis=0)
    return (full["e1_o"], full["e2_o"], full["p1_o"], full["p2_o"]), res


def kernel(**inputs):
    out, _ = _run(inputs)
    return out


# revision 19
# speedup vs baseline: 5.4411x; 1.0117x over previous
"""Trainium2 Bass kernel for nn_ClusterGCN (3-layer 2-edge-type GCN + heads).

Strategy (8 NeuronCores, node-parallel):
  - Nodes sharded contiguously: core c owns rows [c*S, (c+1)*S), S = N/8.
  - Per layer, each core holds a replicated node-major bf16 table of
    h_tilde_t = dinv_t * h (one per edge type) in DRAM; edges are sharded by
    dst. Messages h_tilde[src] are fetched with GPSIMD dma_gather (int16
    indices, lo/hi base split for N > 32768) and scatter-added into a
    feature-major accumulator via one-hot matmuls on the PE
    (out[feat, dst_slot] += msg[edge, feat]^T @ onehot[edge, dst_slot]).
  - g_t = dinv_t * (scatter + dinv_t * h) adds the self-loop, then
    h' = BN(g0 @ W0 + g1 @ W1) with batch stats AllReduced across cores.
  - Tables for the next layer are rebuilt feature-major, PE-transposed to
    node-major, and AllGathered.  Heads (tanh/relu/l2norm MLPs) run
    node-sharded at the end.
"""
import math
import numpy as np
import ml_dtypes

import concourse.bacc as bacc
import concourse.bass as bass
import concourse.mybir as mybir
import concourse.tile as tile
from concourse.library_config import mlp as mlp_lib
from concourse.bass_utils import run_bass_kernel_spmd

NCORES = 8
D = 128
L = 3
EPS_BN = 1e-5
EPS_NORM = 1e-12
IDX_LIMIT = 32768
PIECE = 1024          # gather slots per dma_gather instruction
SENT_DST = 320.0      # sentinel dst slot (bf16-exact, >= 128)

f32 = mybir.dt.float32
bf16 = mybir.dt.float16  # (fp16 everywhere: 8x finer mantissa than bf16, same HW rates)
i16 = mybir.dt.int16
AF = mybir.ActivationFunctionType
OP = mybir.AluOpType


# ---------------------------------------------------------------- host prep

def _prep_type(edge_index, N, S):
    """Per edge type: degrees + per-core common-shape gather/scatter schedule."""
    src = np.asarray(edge_index[0], np.int64)
    dst = np.asarray(edge_index[1], np.int64)
    deg = np.bincount(dst, minlength=N).astype(np.float32) + 1.0
    dinv = (1.0 / np.sqrt(deg)).astype(np.float32)

    NW = (S + 127) // 128
    HI_BASE = N - IDX_LIMIT  # hi-region table base; rows [HI_BASE, N)
    # src in [0, IDX_LIMIT) reachable from region 0; [HI_BASE, N) from region 1.
    # srcs in the overlap [max(HI_BASE,0), IDX_LIMIT) are flexible - used to
    # round region-0 groups up to full chunks and minimize sentinel padding.
    cores = []
    for c in range(NCORES):
        m = (dst >= c * S) & (dst < (c + 1) * S)
        s_c = src[m]
        dl = dst[m] - c * S
        w = dl // 128
        order = np.lexsort((s_c, dl, w))
        cores.append((s_c[order], dl[order], w[order]))

    if N > IDX_LIMIT:
        # chunk counts per (region, window): region 0 gets
        # K0 = max_c ceil(must_lo/128); flexible srcs fill region 0 up to
        # K0*128, remainder goes to region 1.
        K = np.zeros((2, NW), np.int64)
        must_lo = []
        for (s_c, dl, w) in cores:
            cnt_lo = np.bincount(w[s_c < HI_BASE], minlength=NW)
            must_lo.append(cnt_lo)
            K[0] = np.maximum(K[0], (cnt_lo + 127) // 128)
        K[0] = np.maximum(K[0], 1)
        core_reg = []
        for ci, (s_c, dl, w) in enumerate(cores):
            reg = (s_c >= IDX_LIMIT).astype(np.int64)
            for w_ in range(NW):
                cap = K[0][w_] * 128
                flex = np.flatnonzero((w == w_) & (s_c >= HI_BASE) & (s_c < IDX_LIMIT))
                take = min(max(cap - int(must_lo[ci][w_]), 0), len(flex))
                reg[flex[:take]] = 0
                reg[flex[take:]] = 1
            cnt_hi = np.bincount(w[reg == 1], minlength=NW)
            K[1] = np.maximum(K[1], (cnt_hi + 127) // 128)
            core_reg.append(reg)
        K[1] = np.maximum(K[1], 1)
        cores = [(s_c, dl, w, core_reg[ci]) for ci, (s_c, dl, w) in enumerate(cores)]
    else:
        K = np.zeros((2, NW), np.int64)
        for ci, (s_c, dl, w) in enumerate(cores):
            cnt = np.bincount(w, minlength=NW)
            K[0] = np.maximum(K[0], (cnt + 127) // 128)
        K[0] = np.maximum(K[0], 1)
        cores = [(s_c, dl, w, np.zeros(len(s_c), np.int64)) for (s_c, dl, w) in cores]

    schedule = []  # (region, window, nchunks) in slot order
    for r in (0, 1):
        for w_ in range(NW):
            if K[r][w_] > 0:
                schedule.append((r, int(w_), int(K[r][w_])))
    nchunks = sum(k for _, _, k in schedule)
    stot = nchunks * 128

    idx_all = np.zeros((NCORES, max(stot, 128)), np.int64)
    rel_all = np.full((NCORES, max(nchunks, 1) * 128), SENT_DST, np.float64)
    for ci, (s_c, dl, w, reg) in enumerate(cores):
        pos = 0
        for (r, w_, k) in schedule:
            m = (reg == r) & (w == w_)
            n = int(m.sum())
            sv = s_c[m]
            idx_all[ci, pos:pos + n] = sv if r == 0 else sv - (N - IDX_LIMIT)
            rel_all[ci, pos:pos + n] = dl[m] - w_ * 128
            pos += k * 128

    # wrapped int16 index layout: idxs[p, s] = idx[s*16 + p%16]
    cols = max(stot // 16, 1)
    idx_w = np.zeros((NCORES, 128, cols), np.int16)
    if stot:
        a = idx_all[:, :stot].reshape(NCORES, cols, 16)  # [c, s, j]
        for p in range(128):
            idx_w[:, p, :] = a[:, :, p % 16]
    # dst-slot tile: rel[p, chunk] = slot of edge chunk*128+p
    rel_t = np.ascontiguousarray(
        rel_all[:, :nchunks * 128].reshape(NCORES, nchunks, 128).transpose(0, 2, 1)
    ).astype(np.float32)

    return dinv, schedule, idx_w, rel_t, stot, nchunks


def _pieces(schedule):
    """Split slot range into gather pieces that do not cross the lo/hi boundary.
    Returns list of (slot_start, slot_count, region)."""
    out = []
    for r in (0, 1):
        lo = sum(k * 128 for (rr, _, k) in schedule if rr < r)
        n = sum(k * 128 for (rr, _, k) in schedule if rr == r)
        p = lo
        while p < lo + n:
            c = min(PIECE, lo + n - p)
            out.append((p, c, r))
            p += c
    return out


# ---------------------------------------------------------------- device build

def _build(N, S, sch0, stot0, nch0, sch1, stot1, nch1):
    NW = (S + 127) // 128
    NF = (S + 511) // 512  # 512-wide node tiles
    nc = bacc.Bacc("TRN2", target_bir_lowering=False, debug=False,
                   num_devices=NCORES)

    def din(name, shape, dt):
        return nc.dram_tensor(name, shape, dt, kind="ExternalInput")

    tab_in = [din("tab0_in", [N, D], bf16), din("tab1_in", [N, D], bf16)]
    xT_in = din("xT_in", [128, S], f32)
    dinv_in = [din("dinv0_in", [128, S], bf16), din("dinv1_in", [128, S], bf16)]
    idx_in = [din("idx0_in", [128, max(stot0 // 16, 1)], i16),
              din("idx1_in", [128, max(stot1 // 16, 1)], i16)]
    rel_in = [din("rel0_in", [128, max(nch0, 1)], f32),
              din("rel1_in", [128, max(nch1, 1)], f32)]
    wd_in = din("wd_in", [L * 2 * 128, D], bf16)
    gb_in = din("gb_in", [128, 2 * L], f32)
    wh_in = din("wh_in", [6 * 128, D], bf16)
    hb_in = din("hb_in", [128, 6], f32)
    iota_in = din("iota_in", [128, 128], bf16)
    ident_in = din("ident_in", [128, 128], bf16)
    ones_in = din("ones_in", [128, 128], bf16)

    outs = [nc.dram_tensor(n, [128, S], f32, kind="ExternalOutput")
            for n in ("e1_o", "e2_o", "p1_o", "p2_o")]

    with tile.TileContext(nc) as tc:
        with (
            tc.tile_pool(name="const", bufs=1) as const,
            tc.tile_pool(name="g", bufs=1) as gpool,
            tc.tile_pool(name="msg", bufs=4) as msgp,
            tc.tile_pool(name="oh", bufs=24) as ohp,
            tc.tile_pool(name="scr", bufs=1) as scp,
            tc.tile_pool(name="psA", bufs=4, space="PSUM") as psA,
            tc.tile_pool(name="psT", bufs=2, space="PSUM") as psT,
            tc.tile_pool(name="psB", bufs=2, space="PSUM") as psB,
            tc.tile_pool(name="dram", bufs=1, space="DRAM") as dram,
        ):
            nc.gpsimd.load_library(mlp_lib)

            # ---- persistent SBUF tiles
            iota_t = const.tile([128, 128], bf16)
            ident_t = const.tile([128, 128], bf16)
            ones_t = const.tile([128, 128], bf16)
            dinv_t = [const.tile([128, S], bf16, tag=f"dinv{t}", name=f"dinv{t}") for t in (0, 1)]
            idx_t = [const.tile([128, max(stot0 // 16, 1)], i16, tag="idx0", name="idx0"),
                     const.tile([128, max(stot1 // 16, 1)], i16, tag="idx1", name="idx1")]
            rel_t = [const.tile([128, max(nch0, 1)], f32, tag="rel0", name="rel0"),
                     const.tile([128, max(nch1, 1)], f32, tag="rel1", name="rel1")]
            wd_t = const.tile([128, L * 2, D], bf16)     # dense weights
            wh_t = const.tile([128, 6, D], bf16)         # head weights
            gb_t = const.tile([128, 2 * L], f32)
            hb_t = const.tile([128, 6], f32)

            h_t = gpool.tile([128, S], f32, tag="h")
            h16_t = gpool.tile([128, S], bf16, tag="h16")
            gbf_t = [gpool.tile([128, S], bf16, tag=f"gbf{t}", name=f"gbf{t}") for t in (0, 1)]

            nc.sync.dma_start(iota_t[:], iota_in[:])
            nc.sync.dma_start(ident_t[:], ident_in[:])
            nc.sync.dma_start(ones_t[:], ones_in[:])
            for t in (0, 1):
                nc.sync.dma_start(dinv_t[t][:], dinv_in[t][:])
                nc.sync.dma_start(idx_t[t][:], idx_in[t][:])
                nc.sync.dma_start(rel_t[t][:], rel_in[t][:])
            nc.sync.dma_start(
                wd_t[:], wd_in[:].rearrange("(k p) d -> p k d", p=128))
            nc.sync.dma_start(
                wh_t[:], wh_in[:].rearrange("(k p) d -> p k d", p=128))
            nc.sync.dma_start(gb_t[:], gb_in[:])
            nc.sync.dma_start(hb_t[:], hb_in[:])
            nc.sync.dma_start(h_t[:], xT_in[:])
            nc.vector.tensor_copy(out=h16_t[:], in_=h_t[:])

            # ---- internal DRAM for collectives
            ag_in = {}
            ag_out = {}
            for l in (0, 1):
                for t in (0, 1):
                    ag_in[(l, t)] = dram.tile([S, D], bf16, tag=f"agi{l}{t}", name=f"agi{l}{t}")
                    ag_out[(l, t)] = dram.tile([N, D], bf16,
                                               addr_space="Shared",
                                               tag=f"ago{l}{t}",
                                               name=f"ago{l}{t}")
            st_in = [dram.tile([128, 2], f32, tag=f"sti{l}", name=f"sti{l}") for l in range(L)]
            st_out = [dram.tile([128, 2], f32, addr_space="Shared",
                                tag=f"sto{l}", name=f"sto{l}") for l in range(L)]

            schs = (sch0, sch1)
            rg = [list(range(NCORES))]

            for l in range(L):
                # ---------------- scatter phase (both edge types)
                for t in (0, 1):
                    # g init: self-loop inner term  g = dinv * h  (fp16, 4x DVE)
                    nc.vector.tensor_tensor(out=gbf_t[t][:], in0=h16_t[:],
                                            in1=dinv_t[t][:], op=OP.mult)
                    if l == 0:
                        tab_lo = tab_in[t][:]
                        tab_hi = tab_in[t][N - IDX_LIMIT:] if N > IDX_LIMIT else None
                    else:
                        tab_lo = ag_out[(l - 1, t)][:]
                        tab_hi = ag_out[(l - 1, t)][N - IDX_LIMIT:] \
                            if N > IDX_LIMIT else None

                    sch = schs[t]
                    # chunk meta: (region, window, win_first, win_last,
                    #              group_first, group_last); groups = 4 windows
                    # of one region sharing a [128,512] PSUM bank.
                    chunk_meta = []
                    for si, (r, w_, k) in enumerate(sch):
                        gf = (w_ % 4 == 0) or si == 0 or sch[si - 1][0] != r
                        gl = (w_ % 4 == 3) or si == len(sch) - 1 \
                            or sch[si + 1][0] != r
                        for j in range(k):
                            chunk_meta.append(
                                (r, w_, j == 0, j == k - 1,
                                 gf and j == 0, gl and j == k - 1))

                    pieces = _pieces(sch)
                    acc = None
                    for (p0, cnt, r) in pieces:
                        msg = msgp.tile([128, PIECE // 128, 128], bf16,
                                        tag="msg")
                        src_ap = tab_lo if r == 0 else tab_hi
                        nc.gpsimd.dma_gather(
                            msg[:, :cnt // 128, :], src_ap,
                            idx_t[t][:, p0 // 16:(p0 + cnt) // 16],
                            num_idxs=cnt, num_idxs_reg=cnt, elem_size=D,
                        )
                        for ci in range(cnt // 128):
                            gc = p0 // 128 + ci
                            (cr, w_, first, last, gfirst, glast) = chunk_meta[gc]
                            oh = ohp.tile([128, 128], bf16, tag="oh")
                            nc.vector.tensor_scalar(
                                out=oh[:], in0=iota_t[:],
                                scalar1=rel_t[t][:, gc:gc + 1], scalar2=None,
                                op0=OP.is_equal,
                            )
                            if gfirst:
                                acc = psA.tile([128, 512], f32, space="PSUM",
                                               tag="sc")
                            ws = (w_ % 4) * 128
                            nc.tensor.matmul(out=acc[:, ws:ws + 128],
                                             lhsT=msg[:, ci, :],
                                             rhs=oh[:], start=first, stop=last)
                            if glast:
                                base = (w_ // 4) * 512
                                wd = min(512, S - base)
                                nc.vector.tensor_tensor(
                                    out=gbf_t[t][:, base:base + wd],
                                    in0=gbf_t[t][:, base:base + wd],
                                    in1=acc[:, :wd], op=OP.add)
                    # final dst-side scale per 512-tile (lets dense overlap)
                    for ft in range(NF):
                        fsl = slice(ft * 512, min(ft * 512 + 512, S))
                        nc.vector.tensor_tensor(out=gbf_t[t][:, fsl],
                                                in0=gbf_t[t][:, fsl],
                                                in1=dinv_t[t][:, fsl],
                                                op=OP.mult)

                # ---------------- dense + stats partials
                sum_p = scp.tile([128, NF], f32, tag="sump")
                ssq_p = scp.tile([128, NF], f32, tag="ssqp")
                for ft in range(NF):
                    fw = min(512, S - ft * 512)
                    sl = slice(ft * 512, ft * 512 + fw)
                    dp = psB.tile([128, 512], f32, space="PSUM", tag="dense")
                    nc.tensor.matmul(out=dp[:, :fw], lhsT=wd_t[:, l * 2, :],
                                     rhs=gbf_t[0][:, sl], start=True, stop=False)
                    nc.tensor.matmul(out=dp[:, :fw], lhsT=wd_t[:, l * 2 + 1, :],
                                     rhs=gbf_t[1][:, sl], start=False, stop=True)
                    nc.vector.tensor_reduce(out=sum_p[:, ft:ft + 1],
                                            in_=dp[:, :fw],
                                            axis=mybir.AxisListType.X,
                                            op=OP.add)
                    sq = scp.tile([128, 512], f32, tag="sq", bufs=2)
                    nc.scalar.activation(out=sq[:, :fw], in_=dp[:, :fw],
                                         func=AF.Square,
                                         accum_out=ssq_p[:, ft:ft + 1])
                    nc.scalar.activation(out=h_t[:, sl], in_=dp[:, :fw],
                                         func=AF.Copy)

                # ---------------- BN stats allreduce
                st = scp.tile([128, 2], f32, tag="st")
                nc.vector.tensor_reduce(out=st[:, 0:1], in_=sum_p[:],
                                        axis=mybir.AxisListType.X, op=OP.add)
                nc.vector.tensor_reduce(out=st[:, 1:2], in_=ssq_p[:],
                                        axis=mybir.AxisListType.X, op=OP.add)
                nc.sync.dma_start(st_in[l][:], st[:])
                nc.gpsimd.collective_compute(
                    "AllReduce", OP.add, replica_groups=rg,
                    ins=[st_in[l].opt()], outs=[st_out[l].opt()])
                sta = scp.tile([128, 2], f32, tag="sta")
                nc.sync.dma_start(sta[:], st_out[l][:])

                mean = scp.tile([128, 1], f32, tag="mean")
                var = scp.tile([128, 1], f32, tag="var")
                scl = scp.tile([128, 1], f32, tag="scl")
                sht = scp.tile([128, 1], f32, tag="sht")
                tmp = scp.tile([128, 1], f32, tag="tmp1")
                inv_n = 1.0 / float(N)
                nc.vector.tensor_scalar(out=mean[:], in0=sta[:, 0:1],
                                        scalar1=inv_n, scalar2=None, op0=OP.mult)
                nc.vector.tensor_scalar(out=var[:], in0=sta[:, 1:2],
                                        scalar1=inv_n, scalar2=None, op0=OP.mult)
                nc.vector.tensor_tensor(out=tmp[:], in0=mean[:], in1=mean[:],
                                        op=OP.mult)
                nc.vector.tensor_tensor(out=var[:], in0=var[:], in1=tmp[:],
                                        op=OP.subtract)
                # scl = gamma / sqrt(var + eps); sht = beta - mean*scl
                nc.vector.tensor_scalar(out=var[:], in0=var[:], scalar1=EPS_BN,
                                        scalar2=None, op0=OP.add)
                nc.scalar.activation(out=tmp[:], in_=var[:], func=AF.Sqrt)
                nc.vector.reciprocal(out=tmp[:], in_=tmp[:])
                nc.vector.tensor_tensor(out=scl[:], in0=gb_t[:, l:l + 1],
                                        in1=tmp[:], op=OP.mult)
                nc.vector.tensor_tensor(out=tmp[:], in0=mean[:], in1=scl[:],
                                        op=OP.mult)
                nc.vector.tensor_tensor(out=sht[:], in0=gb_t[:, L + l:L + l + 1],
                                        in1=tmp[:], op=OP.subtract)

                # ---------------- normalize (+ relu except last layer)
                nc.scalar.activation(out=h16_t[:], in_=h_t[:],
                                     func=AF.Relu if l < L - 1 else AF.Identity,
                                     bias=sht[:], scale=scl[:])

                # ---------------- next-layer tables + allgather
                if l < L - 1:
                    for t in (0, 1):
                        tb = gbf_t[t]  # reuse as bf16 staging (dense done)
                        nc.vector.tensor_tensor(out=tb[:], in0=h16_t[:],
                                                in1=dinv_t[t][:], op=OP.mult)
                        for w_ in range(NW):
                            wd = min(128, S - w_ * 128)
                            sl = slice(w_ * 128, w_ * 128 + wd)
                            tp = psT.tile([128, 128], bf16, space="PSUM",
                                          tag="tp")
                            nc.tensor.transpose(out=tp[:wd, :], in_=tb[:, sl],
                                                identity=ident_t[:])
                            tsb = scp.tile([128, 128], bf16, tag="tsb", bufs=3)
                            nc.scalar.activation(out=tsb[:wd, :],
                                                 in_=tp[:wd, :], func=AF.Copy)
                            nc.sync.dma_start(ag_in[(l, t)][sl], tsb[:wd, :])
                        nc.gpsimd.collective_compute(
                            "AllGather", OP.bypass, replica_groups=rg,
                            ins=[ag_in[(l, t)].opt()],
                            outs=[ag_out[(l, t)].opt()])

            # ---------------- heads
            def l2norm(dst_t_, x_t_, fw):
                sqb = scp.tile([128, 512], bf16, tag="sqb")
                nc.scalar.activation(out=sqb[:, :fw], in_=x_t_[:, :fw],
                                     func=AF.Square)
                nsq = psB.tile([128, 512], f32, space="PSUM", tag="dense")
                nc.tensor.matmul(out=nsq[:, :fw], lhsT=ones_t[:],
                                 rhs=sqb[:, :fw], start=True, stop=True)
                nrm = scp.tile([128, 512], f32, tag="nrm")
                nc.scalar.activation(out=nrm[:, :fw], in_=nsq[:, :fw],
                                     func=AF.Sqrt)
                nc.vector.tensor_scalar(out=nrm[:, :fw], in0=nrm[:, :fw],
                                        scalar1=EPS_NORM, scalar2=None,
                                        op0=OP.max)
                nc.vector.reciprocal(out=nrm[:, :fw], in_=nrm[:, :fw])
                nc.vector.tensor_tensor(out=dst_t_[:, :fw], in0=x_t_[:, :fw],
                                        in1=nrm[:, :fw], op=OP.mult)

            for ft in range(NF):
                fw = min(512, S - ft * 512)
                sl = slice(ft * 512, ft * 512 + fw)
                # e1 branch
                e1p = psB.tile([128, 512], f32, space="PSUM", tag="dense")
                nc.tensor.matmul(out=e1p[:, :fw], lhsT=wh_t[:, 0, :],
                                 rhs=h16_t[:, sl], start=True, stop=True)
                e1s = scp.tile([128, 512], f32, tag="e1s")
                nc.scalar.activation(out=e1s[:, :fw], in_=e1p[:, :fw],
                                     func=AF.Tanh, bias=hb_t[:, 0:1])
                nc.sync.dma_start(outs[0][:, sl], e1s[:, :fw])
                e1b = scp.tile([128, 512], bf16, tag="e1b")
                nc.vector.tensor_copy(out=e1b[:, :fw], in_=e1s[:, :fw])
                r1p = psB.tile([128, 512], f32, space="PSUM", tag="dense")
                nc.tensor.matmul(out=r1p[:, :fw], lhsT=wh_t[:, 2, :],
                                 rhs=e1b[:, :fw], start=True, stop=True)
                r1b = scp.tile([128, 512], bf16, tag="r1b")
                nc.scalar.activation(out=r1b[:, :fw], in_=r1p[:, :fw],
                                     func=AF.Relu, bias=hb_t[:, 2:3])
                z1p = psB.tile([128, 512], f32, space="PSUM", tag="dense")
                nc.tensor.matmul(out=z1p[:, :fw], lhsT=wh_t[:, 3, :],
                                 rhs=r1b[:, :fw], start=True, stop=True)
                z1s = scp.tile([128, 512], f32, tag="z1s")
                nc.scalar.activation(out=z1s[:, :fw], in_=z1p[:, :fw],
                                     func=AF.Identity, bias=hb_t[:, 3:4])
                p1s = scp.tile([128, 512], f32, tag="p1s")
                l2norm(p1s, z1s, fw)
                nc.sync.dma_start(outs[2][:, sl], p1s[:, :fw])

                # e2 branch
                e2p = psB.tile([128, 512], f32, space="PSUM", tag="dense")
                nc.tensor.matmul(out=e2p[:, :fw], lhsT=wh_t[:, 1, :],
                                 rhs=h16_t[:, sl], start=True, stop=True)
                t2s = scp.tile([128, 512], f32, tag="t2s")
                nc.scalar.activation(out=t2s[:, :fw], in_=e2p[:, :fw],
                                     func=AF.Tanh, bias=hb_t[:, 1:2])
                e2s = scp.tile([128, 512], f32, tag="e2s")
                l2norm(e2s, t2s, fw)
                nc.sync.dma_start(outs[1][:, sl], e2s[:, :fw])
                e2b = scp.tile([128, 512], bf16, tag="e2b")
                nc.vector.tensor_copy(out=e2b[:, :fw], in_=e2s[:, :fw])
                r2p = psB.tile([128, 512], f32, space="PSUM", tag="dense")
                nc.tensor.matmul(out=r2p[:, :fw], lhsT=wh_t[:, 4, :],
                                 rhs=e2b[:, :fw], start=True, stop=True)
                r2b = scp.tile([128, 512], bf16, tag="r2b")
                nc.scalar.activation(out=r2b[:, :fw], in_=r2p[:, :fw],
                                     func=AF.Relu, bias=hb_t[:, 4:5])
                z2p = psB.tile([128, 512], f32, space="PSUM", tag="dense")
                nc.tensor.matmul(out=z2p[:, :fw], lhsT=wh_t[:, 5, :],
                                 rhs=r2b[:, :fw], start=True, stop=True)
                z2s = scp.tile([128, 512], f32, tag="z2s")
                nc.scalar.activation(out=z2s[:, :fw], in_=z2p[:, :fw],
                                     func=AF.Identity, bias=hb_t[:, 5:6])
                p2s = scp.tile([128, 512], f32, tag="p2s")
                l2norm(p2s, z2s, fw)
                nc.sync.dma_start(outs[3][:, sl], p2s[:, :fw])

    nc.compile()
    return nc


# ---------------------------------------------------------------- entry point

def _run(inputs, trace=False, trace_kwargs=None, nc_out=None):
    x = np.asarray(inputs["x"], np.float32)
    N = x.shape[0]
    assert N % NCORES == 0
    S = N // NCORES

    dinv0, sch0, idx0, rel0, stot0, nch0 = _prep_type(inputs["edge_index0"], N, S)
    dinv1, sch1, idx1, rel1, stot1, nch1 = _prep_type(inputs["edge_index1"], N, S)

    nc = _build(N, S, sch0, stot0, nch0, sch1, stot1, nch1)
    if nc_out is not None:
        nc_out.append(nc)

    tab0 = (x * dinv0[:, None]).astype(np.float16)
    tab1 = (x * dinv1[:, None]).astype(np.float16)

    W0 = np.asarray(inputs["W0"], np.float32)
    W1 = np.asarray(inputs["W1"], np.float32)
    wd = np.zeros((L * 2 * 128, D), np.float32)
    for l in range(L):
        wd[(l * 2) * 128:(l * 2 + 1) * 128] = W0[l]
        wd[(l * 2 + 1) * 128:(l * 2 + 2) * 128] = W1[l]
    gb = np.stack([np.asarray(inputs["gamma"], np.float32).T,
                   np.asarray(inputs["beta"], np.float32).T], 0)
    gb = np.concatenate([gb[0], gb[1]], axis=1)  # [128, 2L]
    wh = np.concatenate([np.asarray(inputs[k], np.float32) for k in
                         ("emb1_W", "emb2_W", "ph1_Wa", "ph1_Wb",
                          "ph2_Wa", "ph2_Wb")], 0)
    hb = np.stack([np.asarray(inputs[k], np.float32) for k in
                   ("emb1_b", "emb2_b", "ph1_ba", "ph1_bb",
                    "ph2_ba", "ph2_bb")], 1)

    iota = np.broadcast_to(np.arange(128, dtype=np.float32),
                           (128, 128)).astype(np.float16)
    ident = np.eye(128, dtype=np.float16)
    ones = np.ones((128, 128), np.float16)

    in_maps = []
    for c in range(NCORES):
        sl = slice(c * S, (c + 1) * S)
        in_maps.append({
            "tab0_in": tab0, "tab1_in": tab1,
            "xT_in": np.ascontiguousarray(x[sl].T),
            "dinv0_in": np.ascontiguousarray(
                np.broadcast_to(dinv0[sl], (128, S))).astype(np.float16),
            "dinv1_in": np.ascontiguousarray(
                np.broadcast_to(dinv1[sl], (128, S))).astype(np.float16),
            "idx0_in": idx0[c], "idx1_in": idx1[c],
            "rel0_in": rel0[c], "rel1_in": rel1[c],
            "wd_in": wd.astype(np.float16),
            "gb_in": gb, "wh_in": wh.astype(np.float16), "hb_in": hb,
            "iota_in": iota, "ident_in": ident, "ones_in": ones,
        })

    res = run_bass_kernel_spmd(nc, in_maps, list(range(NCORES)),
                               trace=trace, **(trace_kwargs or {}))

    full = {}
    for name in ("e1_o", "e2_o", "p1_o", "p2_o"):
        full[name] = np.concatenate(
            [res.results[c][name].T for c in range(NCORES)], axis=0)
    return (full["e1_o"], full["e2_o"], full["p1_o"], full["p2_o"]), res


def kernel(**inputs):
    out, _ = _run(inputs)
    return out
